# revision 10
# baseline (speedup 1.0000x reference)
"""Trainium2 Bass kernel for nn_MFA_87067577025371.

Architecture (B=2, C=64, Ci=32, H=W=96, N=9216):
  k,v = 1x1conv(xA); q = 1x1conv(xB)
  A   = softmax(v^T q, axis=2)            # [B, N, N], softmax over query dim m
  av  = k @ A                             # [B, Ci, N]
  out = relu(BN2(Wo @ BN1(Wg @ av)) + xB)

Sharding: (batch, key-row chunk) across 8 cores — each core owns 2304 rows
of the score matrix for one batch, computes E = exp(s)/16 for all m,
row-sums Z' = Z/16, scales kT by 32/Z' and accumulates its partial
av*32 = kts^T E (the uniform 1/16 E scale and x32 kts scale cancel in
softmax normalization; the tail copies rescale by 1/32).  A second tiny
launch applies the host-folded epilogue per (batch, query chunk).

Speed structure (v4):
  * both score projections fold on host into M = Wv_aug^T Wq_aug/2, so
    s/16 = vv^T xBd with vv = M^T xA (tiny on-device projection, fp8) and
    xBd = xB/8 quantized to fp8 on host (layout prep only).  Score
    matmuls run fp8e4 DoubleRow at 0.5 PE-cycles/column.
  * exp splits across engines: ~4.25 strips/block on ACT via
    activation(Exp, scale=16, bias=-ln16, accum_out=Zpartial), the rest
    on DVE via a custom op EXP16 = (c0+u(c1+u c2))^16 with the 1/16
    folded into the coefficients.  End-to-end ~1.7e-3 rel err.
  * E column region [0:6144] is stored fp8 in block-PAIR tiles
    [128,2,6144] so av matmuls for that region run DoubleRow pairs (2
    blocks per instruction); region [6144:9216] stays bf16 (DVE needs
    2-byte dtype for its fast row-sum pass on the strips it exp'd).
  * av spreads through av_acc with region-wise groups (ACT region: two
    groups of 8, all-pair emits; DVE region: 5/5/6) and a direct block
    pair (16,17) folded into the tail reduce.
"""

import os
import sys

import numpy as np

for _p in ("/opt/trn_rl_repo", "/root/.axon_site/_ro/trn_rl_repo"):
    if os.path.isdir(_p) and _p not in sys.path:
        sys.path.insert(0, _p)

import ml_dtypes  # noqa: E402

BF16 = ml_dtypes.bfloat16
FP8 = ml_dtypes.float8_e4m3fn

# ---- problem constants (hardcoded per contract) ----
B, C, CI, H, W = 2, 64, 32, 96, 96
N = H * W                  # 9216
NCORES = 8
NCHUNK = N // 4            # 2304 key rows per core
NSUB = NCHUNK // 128       # 18 blocks of 128 rows
STRIP = 1536               # exp strip (3 PSUM banks)
NSTRIP = N // STRIP        # 6
AVS = 512                  # av matmul strip
NAVS = N // AVS            # 18
CAUG = C + 1               # 65 (bias row folded in)
KD = 33                    # dual-layout contraction rows (65 ch + pad)/2
EPS = 1e-5
LN16 = float(np.log(16.0))

RSPLIT = 12                # av strips 0..11 = fp8 pair region, 12..17 = bf16
GA = [(0, 8), (8, 6)]      # ACT-region spread groups (block ranges)
GD = [(0, 5), (5, 5), (10, 6)]  # DVE-region spread groups
DIRECT_A = (14, 4)         # blocks 14..17 (pairs p7, p8): direct in the tail
DIRECT_D = (16, 2)         # blocks 16,17: direct in the tail

# strips < dve_from(j) -> ACT exp; else DVE EXP16.  First blocks lean on
# ACT while DVE does the vv/kT setup copies.
DVE_FROM_LIST = [5, 5] + [4] * 16

# EXP16 poly: exp(16u)/16 ~ (g*c0 + u*(g*c1 + u*g*c2))^16, g = 16^(-1/16)
_G = 16.0 ** (-1.0 / 16.0)
EXPC = (0.9999280385484721 * _G, 1.000676421773311 * _G, 0.5251343712954386 * _G)

_CACHE = {}


def _register_exp16():
    """Register the EXP16 custom DVE op (idempotent)."""
    import concourse.dve_ops as do
    from concourse.dve_spec import Spec, Src0, C0, C1, C2, sq, lower, _has_src1
    from concourse.dve_uop import DveOpSpec

    for op in do.OPS:
        if op.name == "EXP16_ANT":
            return op
    u = Src0
    p = C2 + u * (C0 + u * C1)
    body = sq(sq(sq(sq(p))))

    def ref(in0, in1, c0, c1, c2):
        x = in0.astype(np.float32)
        pp = np.float32(c2) + x * (np.float32(c0) + x * np.float32(c1))
        r = pp * pp
        r = r * r
        r = r * r
        return r * r

    spec = Spec(body=body, reference=ref)
    row = max(do._SUB_OPCODE_FOR_NAME.values()) + 1
    assert row < 0x20, row
    shas = {}
    for ver in ("v3", "v4"):
        s = DveOpSpec(name="EXP16_ANT", opcode=row, uops=lower(spec, ver=ver),
                      rd1_en=_has_src1(spec))
        shas[ver] = s.sha(ver)
    op = do.DveOp("EXP16_ANT", spec, subdim=False, uops_sha=shas)
    do.OPS.append(op)
    do._SUB_OPCODE_FOR_NAME["EXP16_ANT"] = row
    do.CUSTOM_DVE_SPECS["EXP16_ANT"] = spec
    return op


def _build_phase1():
    import concourse.bacc as bacc
    import concourse.tile as tile
    from concourse import mybir

    exp16 = _register_exp16()

    f32 = mybir.dt.float32
    bf16 = mybir.dt.bfloat16
    fp8 = mybir.dt.float8e4
    AX = mybir.AxisListType
    AF = mybir.ActivationFunctionType
    ADD = mybir.AluOpType.add
    DR = mybir.MatmulPerfMode.DoubleRow

    nc = bacc.Bacc("TRN2", target_bir_lowering=False, debug=False)

    xBd_d = nc.dram_tensor("xBd", [KD, 2, N], fp8, kind="ExternalInput").ap()
    xA_aug_d = nc.dram_tensor("xA_aug", [CAUG, NCHUNK], bf16, kind="ExternalInput").ap()
    mv_d = nc.dram_tensor("mv", [CAUG, 2 * KD], bf16, kind="ExternalInput").ap()
    wk_d = nc.dram_tensor("wk", [CAUG, CI], bf16, kind="ExternalInput").ap()
    omap_d = nc.dram_tensor("omap", [128, CI], bf16, kind="ExternalInput").ap()
    avp_d = nc.dram_tensor("av_part", [CI, N], bf16, kind="ExternalOutput").ap()

    # region-wise group lookup: which group does block j belong to
    ga_of, gd_of = {}, {}
    for gi, (g0, ng) in enumerate(GA):
        for j in range(g0, g0 + ng):
            ga_of[j] = gi
    for gi, (g0, ng) in enumerate(GD):
        for j in range(g0, g0 + ng):
            gd_of[j] = gi

    with tile.TileContext(nc) as tc:
        with (
            tc.tile_pool(name="big", bufs=1) as big,
            tc.tile_pool(name="pers", bufs=1) as pers,
            tc.tile_pool(name="small", bufs=4) as small,
            tc.tile_pool(name="stats", bufs=8) as stats,
            tc.tile_pool(name="scp", bufs=2, space="PSUM") as scp,
            tc.tile_pool(name="avp", bufs=2, space="PSUM") as avp,
        ):
            # ---- warmup: ACT exp-table load + bias const before data ----
            warm = small.tile([128, 1], f32, tag="warm")
            nc.vector.memset(warm[:, :], 0.0)
            bln = small.tile([128, 1], f32, tag="bln", bufs=1)
            nc.vector.memset(bln[:, :], -LN16)
            warm2 = small.tile([128, 1], f32, tag="warm")
            nc.scalar.activation(warm2[:, :], warm[:, :], AF.Exp)

            # ---- input DMAs: mv + xA first chunk gate the vv sliver ----
            mv_sb = small.tile([CAUG, 2 * KD], bf16, tag="w")
            nc.sync.dma_start(mv_sb[:], mv_d[:])
            xA_sb = pers.tile([CAUG, NCHUNK], bf16, tag="xA")
            nc.sync.dma_start(xA_sb[:, 0:1152], xA_aug_d[:, 0:1152])
            xBd_sb = pers.tile([KD, 2, N], fp8, tag="xbd")
            nc.sync.dma_start(xBd_sb[:, :, 0:1536], xBd_d[:, :, 0:1536])
            nc.sync.dma_start(xA_sb[:, 1152:2304], xA_aug_d[:, 1152:2304])
            wk_sb = small.tile([CAUG, CI], bf16, tag="w")
            nc.gpsimd.dma_start(wk_sb[:], wk_d[:])
            omap_sb = small.tile([128, CI], bf16, tag="w")
            nc.gpsimd.dma_start(omap_sb[:], omap_d[:])
            for blk in range(1, NSTRIP):
                lo, hi = blk * 1536, (blk + 1) * 1536
                nc.sync.dma_start(xBd_sb[:, :, lo:hi], xBd_d[:, :, lo:hi])

            vv_sb = pers.tile([KD, 2, NCHUNK], fp8, tag="vv")
            kT_sb = pers.tile([128, NSUB * CI], bf16, tag="kT")
            av_acc = pers.tile([128, N], bf16, tag="avacc")

            # ---- vv sliver [*,0:128] for both halves: unblocks block 0 ----
            for h in range(2):
                pt = scp.tile([128, STRIP], f32, tag="sc")
                nc.tensor.matmul(
                    pt[0:KD, 0:128], mv_sb[:, h * KD:(h + 1) * KD],
                    xA_sb[:, 0:128], start=True, stop=True,
                )
                nc.vector.tensor_copy(vv_sb[:, h, 0:128], pt[0:KD, 0:128])

            # deferred setup work, one item per early exp-strip slot:
            # ('vv', h, base, width) or ('kt', half)
            setup_q = [("kt", 0), ("kt", 1)]
            for cbase, cw in ((128, 512), (640, 512), (1152, 512),
                              (1664, 512), (2176, 128)):
                for h in range(2):
                    setup_q.append(("vv", h, cbase, cw))

            def do_setup(item):
                if item[0] == "kt":
                    half = item[1]
                    pt = avp.tile([128, AVS], f32, tag="av")
                    for i, j in enumerate(range(9 * half, 9 * (half + 1))):
                        nc.tensor.matmul(
                            pt[:, i * CI:(i + 1) * CI],
                            xA_sb[:, j * 128:(j + 1) * 128],
                            wk_sb[:, :], start=True, stop=True,
                        )
                    nc.vector.tensor_copy(
                        kT_sb[:, half * 9 * CI:(half + 1) * 9 * CI],
                        pt[:, 0:9 * CI])
                else:
                    _, h, cbase, cw = item
                    pt = avp.tile([128, AVS], f32, tag="av")
                    nc.tensor.matmul(
                        pt[0:KD, 0:cw], mv_sb[:, h * KD:(h + 1) * KD],
                        xA_sb[:, cbase:cbase + cw], start=True, stop=True,
                    )
                    nc.vector.tensor_copy(
                        vv_sb[:, h, cbase:cbase + cw], pt[0:KD, 0:cw])

            # ---- main loop ----
            pair_tiles = [None] * (NSUB // 2)   # fp8 [128, 2, 6144]
            edve_tiles = [None] * NSUB          # bf16 [128, 3072]
            kts8_tiles = [None] * (NSUB // 2)   # fp8 [128, 2, CI] (x32 scale)
            kts16_tiles = [None] * NSUB         # bf16 [128, CI]   (x32 scale)
            av_queue = []   # ('A'|'D', group_index, strip)
            emitted = [0]
            slot_no = [0]

            def emit_av(region, gi, t):
                at = avp.tile([128, AVS], f32, tag="av")
                csl = slice(t * AVS, (t + 1) * AVS)
                if region == "A":
                    g0, ng = GA[gi]
                    npair = ng // 2
                    for p in range(npair):
                        pi = g0 // 2 + p
                        nc.tensor.matmul(
                            at[0:32, :],
                            kts8_tiles[pi][:, :, :],
                            pair_tiles[pi][:, :, csl],
                            start=p == 0, stop=p == npair - 1,
                            perf_mode=DR,
                        )
                    rows = 32
                else:
                    g0, ng = GD[gi]
                    for cg in range(ng):
                        pos = cg % 4
                        nc.tensor.matmul(
                            at[pos * 32:(pos + 1) * 32, :],
                            kts16_tiles[g0 + cg][:, :],
                            edve_tiles[g0 + cg][:, (t - RSPLIT) * AVS:
                                                (t - RSPLIT + 1) * AVS],
                            start=cg == pos, stop=cg + 4 >= ng,
                            tile_position=(0, pos * 32),
                        )
                    rows = min(ng, 4) * 32
                dst = av_acc[0:rows, csl]
                src = at[0:rows, :]
                if gi == 0:
                    nc.vector.tensor_copy(dst, src)
                else:
                    nc.vector.tensor_tensor(dst, dst, src, op=ADD)
                emitted[0] += 1

            total_slots = NSUB * NSTRIP

            for j in range(NSUB):
                if j % 2 == 0:
                    pair_tiles[j // 2] = big.tile(
                        [128, 2, RSPLIT * AVS], fp8, tag="epair", bufs=5,
                        name=f"epair{j // 2}")
                epair = pair_tiles[j // 2]
                edve = big.tile([128, N - RSPLIT * AVS], bf16, tag="edve", bufs=8)
                edve_tiles[j] = edve
                zp = stats.tile([128, 8], f32, tag="zp")
                dve_from = DVE_FROM_LIST[j]
                # DVE strips first: their exp overlaps this block's ACT strips
                for s in list(range(dve_from, NSTRIP)) + list(range(dve_from)):
                    sc = scp.tile([128, STRIP], f32, tag="sc")
                    for t3 in range(STRIP // 512):
                        col = s * STRIP + t3 * 512
                        nc.tensor.matmul(
                            sc[:, t3 * 512:(t3 + 1) * 512],
                            vv_sb[:, :, j * 128:(j + 1) * 128],
                            xBd_sb[:, :, col:col + 512],
                            start=True, stop=True,
                            perf_mode=DR,
                        )
                    if s < 4:
                        e_dst = epair[:, j % 2, s * STRIP:(s + 1) * STRIP]
                    else:
                        e_dst = edve[:, (s - 4) * STRIP:(s - 3) * STRIP]
                    if s < dve_from:
                        nc.scalar.activation(
                            e_dst, sc[:, :], AF.Exp,
                            scale=16.0, bias=bln[:, :],
                            accum_out=zp[:, s:s + 1],
                        )
                    else:
                        nc.vector._custom_dve(
                            exp16, out=e_dst, in0=sc[:, :],
                            s0=EXPC[1], s1=EXPC[2], imm2=EXPC[0],
                        )
                        nc.vector.tensor_scalar(
                            e_dst, e_dst, 1.0, None,
                            op0=mybir.AluOpType.mult, op1=mybir.AluOpType.add,
                            accum_out=zp[:, s:s + 1],
                        )
                    if setup_q:
                        do_setup(setup_q.pop(0))
                    if av_queue:
                        emit_av(*av_queue.pop(0))
                    slot_no[0] += 1
                    slots_left = total_slots - slot_no[0]
                    if av_queue and len(av_queue) > slots_left:
                        emit_av(*av_queue.pop(0))
                z = stats.tile([128, 1], f32, tag="z")
                nc.vector.reduce_sum(z[:, :], zp[:, 0:NSTRIP], axis=AX.X)
                rinv = stats.tile([128, 1], f32, tag="rinv")
                nc.vector.reciprocal(rinv[:, :], z[:, :])
                if j % 2 == 0:
                    kts8_tiles[j // 2] = stats.tile([128, 2, CI], fp8, tag="k8",
                                                    name=f"k8_{j // 2}")
                nc.vector.tensor_scalar(
                    kts8_tiles[j // 2][:, j % 2, :],
                    kT_sb[:, j * CI:(j + 1) * CI],
                    rinv[:, :], 32.0,
                    op0=mybir.AluOpType.mult, op1=mybir.AluOpType.mult,
                )
                kts16 = stats.tile([128, CI], bf16, tag="k16")
                kts16_tiles[j] = kts16
                nc.vector.tensor_scalar(
                    kts16[:, :], kT_sb[:, j * CI:(j + 1) * CI],
                    rinv[:, :], 32.0,
                    op0=mybir.AluOpType.mult, op1=mybir.AluOpType.mult,
                )
                gi = ga_of.get(j)
                if gi is not None and j == GA[gi][0] + GA[gi][1] - 1:
                    av_queue.extend(("A", gi, t) for t in range(RSPLIT))
                gi = gd_of.get(j)
                if gi is not None and j == GD[gi][0] + GD[gi][1] - 1:
                    av_queue.extend(("D", gi, t) for t in range(RSPLIT, NAVS))

            # ---- tail: leftover spread, then fused reduce + direct pair ----
            for it in av_queue:
                emit_av(*it)
            av_out = big.tile([CI, N], bf16, tag="avout", bufs=1)
            for t in range(NAVS):
                pool = scp if t % 2 == 0 else avp
                tag = "sc" if t % 2 == 0 else "av"
                rt = pool.tile([128, AVS], f32, tag=tag)
                csl = slice(t * AVS, (t + 1) * AVS)
                if t < RSPLIT:
                    # ACT region: av_acc rows 0:32 hold the full spread sum;
                    # omap[0:32] is the 32x32 identity.
                    nc.tensor.matmul(
                        rt[0:CI, :], omap_sb[0:32, :], av_acc[0:32, csl],
                        start=True, stop=False,
                    )
                    da0, dan = DIRECT_A
                    for p in range(dan // 2):
                        pi = da0 // 2 + p
                        nc.tensor.matmul(
                            rt[0:CI, :],
                            kts8_tiles[pi][:, :, :],
                            pair_tiles[pi][:, :, csl],
                            start=False, stop=(p == dan // 2 - 1),
                            perf_mode=DR,
                        )
                else:
                    nc.tensor.matmul(
                        rt[0:CI, :], omap_sb[:, :], av_acc[:, csl],
                        start=True, stop=False,
                    )
                    dd0, ddn = DIRECT_D
                    for d in range(ddn):
                        nc.tensor.matmul(
                            rt[0:CI, :],
                            kts16_tiles[dd0 + d][:, :],
                            edve_tiles[dd0 + d][:, (t - RSPLIT) * AVS:
                                                (t - RSPLIT + 1) * AVS],
                            start=False, stop=(d == ddn - 1),
                        )
                dst = av_out[:, csl]
                if t % 2 == 0:
                    nc.scalar.activation(dst, rt[0:CI, :], AF.Copy,
                                         bias=0.0, scale=1.0 / 32.0)
                else:
                    nc.vector.tensor_scalar_mul(dst, rt[0:CI, :], 1.0 / 32.0)
                if t % 3 == 2 or t == NAVS - 1:
                    lo = (t - (t % 3)) * AVS
                    nc.sync.dma_start(
                        avp_d[:, lo:(t + 1) * AVS], av_out[:, lo:(t + 1) * AVS]
                    )

    nc.compile()
    return nc


def _build_phase2():
    import concourse.bacc as bacc
    import concourse.tile as tile
    from concourse import mybir

    f32 = mybir.dt.float32
    bf16 = mybir.dt.bfloat16
    AF = mybir.ActivationFunctionType
    MQ = N // 4   # 2304 output columns per core
    K2 = CI + 1 + C  # 97 contraction rows: WfinT | cfin | I (residual)

    nc = bacc.Bacc("TRN2", target_bir_lowering=False, debug=False)

    avs_d = nc.dram_tensor("avs", [K2, C + MQ], bf16, kind="ExternalInput").ap()
    out_d = nc.dram_tensor("outc", [C, MQ], f32, kind="ExternalOutput").ap()

    with tile.TileContext(nc) as tc:
        with (
            tc.tile_pool(name="sb", bufs=1) as sb,
            tc.tile_pool(name="ps", bufs=4, space="PSUM") as ps,
        ):
            warm = sb.tile([128, 1], f32, tag="warm")
            nc.vector.memset(warm[:, :], 0.0)
            warm2 = sb.tile([128, 1], f32, tag="warm2")
            nc.scalar.activation(warm2[:, :], warm[:, :], AF.Relu)

            av_aug = sb.tile([K2, C + MQ], bf16, tag="avaug")
            nc.sync.dma_start(av_aug[:, 0:C + 512], avs_d[:, 0:C + 512])
            nc.scalar.dma_start(av_aug[:, C + 512:C + 1024], avs_d[:, C + 512:C + 1024])
            nc.sync.dma_start(av_aug[:, C + 1024:C + 1536], avs_d[:, C + 1024:C + 1536])
            nc.scalar.dma_start(av_aug[:, C + 1536:C + 2048], avs_d[:, C + 1536:C + 2048])
            nc.sync.dma_start(av_aug[:, C + 2048:C + MQ], avs_d[:, C + 2048:C + MQ])
            o_sb = sb.tile([C, MQ], f32, tag="o")

            nstr = (MQ + 511) // 512
            for s in range(nstr):
                sw = min(512, MQ - s * 512)
                sl = slice(s * 512, s * 512 + sw)
                op = ps.tile([128, 512], f32, tag="rp")
                nc.tensor.matmul(
                    op[0:C, 0:sw], av_aug[:, 0:C],
                    av_aug[:, C + s * 512:C + s * 512 + sw],
                    start=True, stop=True,
                )
                nc.scalar.activation(o_sb[:, sl], op[0:C, 0:sw], AF.Relu)
                nc.sync.dma_start(out_d[:, sl], o_sb[:, sl])

    nc.compile()
    return nc


def _get_programs():
    if "p1" not in _CACHE:
        _CACHE["p1"] = _build_phase1()
        _CACHE["p2"] = _build_phase2()
    return _CACHE["p1"], _CACHE["p2"]


def _ones_map(dtype):
    m = np.zeros((128, CI), dtype)
    for g in range(4):
        m[g * 32 + np.arange(CI), np.arange(CI)] = 1
    return m


def kernel(xA, xB, Wk, bk, Wv, bv, Wq, bq, Wg,
           g1_gamma, g1_beta, g1_mean, g1_var,
           Wo, bo, g2_gamma, g2_beta, g2_mean, g2_var):
    from concourse.bass_utils import run_bass_kernel_spmd

    p1, p2 = _get_programs()

    xA = np.asarray(xA, np.float32).reshape(B, C, N)
    xB = np.asarray(xB, np.float32).reshape(B, C, N)

    # ---- host-side weight folding (tiny) ----
    s1 = np.asarray(g1_gamma) / np.sqrt(np.asarray(g1_var) + EPS)
    Wg_f = s1[:, None] * np.asarray(Wg)
    c1 = np.asarray(g1_beta) - s1 * np.asarray(g1_mean)
    s2 = np.asarray(g2_gamma) / np.sqrt(np.asarray(g2_var) + EPS)
    Wo_f = s2[:, None] * np.asarray(Wo)
    c2 = s2 * (np.asarray(bo) - np.asarray(g2_mean)) + np.asarray(g2_beta)
    Wfin = (Wo_f @ Wg_f).astype(np.float32)          # [C, CI]
    cfin = (Wo_f @ c1 + c2).astype(np.float32)       # [C]

    # fold both score projections into M = Wv_aug^T Wq_aug / 2 (so that
    # (M^T xA)^T (xB/8) = s/16), padded to 66 columns for the dual layout
    Wv_aug = np.concatenate([np.asarray(Wv), np.asarray(bv)[:, None]], 1)
    Wq_aug = np.concatenate([np.asarray(Wq), np.asarray(bq)[:, None]], 1)
    M = (Wv_aug.T @ Wq_aug / 2.0).astype(np.float32)         # [65, 65]
    mv = np.concatenate([M, np.zeros((CAUG, 1), np.float32)], 1).astype(BF16)
    wk_aug = np.concatenate([np.asarray(Wk).T, np.asarray(bk)[None, :]], 0).astype(BF16)
    omap16 = _ones_map(BF16)

    ones_n = np.ones((1, N), np.float32)

    # xBd: [66, N]/8 -> fp8 dual [33, 2, N] (pure layout prep)
    xBd = []
    for b in range(B):
        xb66 = np.concatenate([xB[b] / 8.0, ones_n / 8.0,
                               np.zeros((1, N), np.float32)], 0)
        xBd.append(np.ascontiguousarray(
            xb66.reshape(2, KD, N).transpose(1, 0, 2)).astype(FP8))

    # ---- phase 1: per-core (batch, key-row chunk) partial attention ----
    in_maps1 = []
    for core in range(NCORES):
        b, chunk = divmod(core, 4)
        sl = slice(chunk * NCHUNK, (chunk + 1) * NCHUNK)
        in_maps1.append({
            "xBd": xBd[b],
            "xA_aug": np.concatenate([xA[b][:, sl], ones_n[:, sl]], 0).astype(BF16),
            "mv": mv, "wk": wk_aug,
            "omap": omap16,
        })
    res1 = run_bass_kernel_spmd(p1, in_maps1, list(range(NCORES)))
    av_parts = [res1.results[i]["av_part"].astype(np.float32) for i in range(NCORES)]

    # ---- phase 2: per-core (batch, query chunk) epilogue ----
    MQ = N // 4
    av_sum = [sum(av_parts[b * 4 + i] for i in range(4)) for b in range(B)]
    wblk = np.concatenate([Wfin.T, cfin[None, :], np.eye(C, dtype=np.float32)], 0)
    ones_mq = np.ones((1, MQ), np.float32)
    in_maps2 = []
    for core in range(NCORES):
        b, mq = divmod(core, 4)
        msl = slice(mq * MQ, (mq + 1) * MQ)
        rhs = np.concatenate([av_sum[b][:, msl], ones_mq, xB[b][:, msl]], 0)
        in_maps2.append({
            "avs": np.concatenate([wblk, rhs], 1).astype(BF16),
        })
    res2 = run_bass_kernel_spmd(p2, in_maps2, list(range(NCORES)))

    out = np.zeros((B, C, N), np.float32)
    for core in range(NCORES):
        b, mq = divmod(core, 4)
        out[b][:, mq * MQ:(mq + 1) * MQ] = res2.results[core]["outc"]
    return out.reshape(B, C, H, W)


# revision 11
# speedup vs baseline: 1.0787x; 1.0787x over previous
"""Trainium2 Bass kernel for nn_MFA_87067577025371.

Architecture (B=2, C=64, Ci=32, H=W=96, N=9216):
  k,v = 1x1conv(xA); q = 1x1conv(xB)
  A   = softmax(v^T q, axis=2)            # [B, N, N], softmax over query dim m
  av  = k @ A                             # [B, Ci, N]
  out = relu(BN2(Wo @ BN1(Wg @ av)) + xB)

Sharding: (batch, key-row chunk) across 8 cores — each core owns 2304 rows
of the score matrix for one batch, computes E = exp(s)/16 for all m,
row-sums Z' = Z/16, scales kT by 32/Z' and accumulates its partial
av*32 = kts^T E (the uniform 1/16 E scale and x32 kts scale cancel in
softmax normalization; the tail copies rescale by 1/32).  A second tiny
launch applies the host-folded epilogue per (batch, query chunk).

Speed structure (v4):
  * both score projections fold on host into M = Wv_aug^T Wq_aug/2, so
    s/16 = vv^T xBd with vv = M^T xA (tiny on-device projection, fp8) and
    xBd = xB/8 quantized to fp8 on host (layout prep only).  Score
    matmuls run fp8e4 DoubleRow at 0.5 PE-cycles/column.
  * exp splits across engines: ~4.25 strips/block on ACT via
    activation(Exp, scale=16, bias=-ln16, accum_out=Zpartial), the rest
    on DVE via a custom op EXP16 = (c0+u(c1+u c2))^16 with the 1/16
    folded into the coefficients.  End-to-end ~1.7e-3 rel err.
  * E column region [0:6144] is stored fp8 in block-PAIR tiles
    [128,2,6144] so av matmuls for that region run DoubleRow pairs (2
    blocks per instruction); region [6144:9216] stays bf16 (DVE needs
    2-byte dtype for its fast row-sum pass on the strips it exp'd).
  * av spreads through av_acc with region-wise groups (ACT region: two
    groups of 8, all-pair emits; DVE region: 5/5/6) and a direct block
    pair (16,17) folded into the tail reduce.
"""

import os
import sys

import numpy as np

for _p in ("/opt/trn_rl_repo", "/root/.axon_site/_ro/trn_rl_repo"):
    if os.path.isdir(_p) and _p not in sys.path:
        sys.path.insert(0, _p)

import ml_dtypes  # noqa: E402

BF16 = ml_dtypes.bfloat16
FP8 = ml_dtypes.float8_e4m3fn

# ---- problem constants (hardcoded per contract) ----
B, C, CI, H, W = 2, 64, 32, 96, 96
N = H * W                  # 9216
NCORES = 8
NCHUNK = N // 4            # 2304 key rows per core
NSUB = NCHUNK // 128       # 18 blocks of 128 rows
STRIP = 1536               # exp strip (3 PSUM banks)
NSTRIP = N // STRIP        # 6
AVS = 512                  # av matmul strip
NAVS = N // AVS            # 18
CAUG = C + 1               # 65 (bias row folded in)
KD = 33                    # dual-layout contraction rows (65 ch + pad)/2
EPS = 1e-5
LN16 = float(np.log(16.0))

RSPLIT = 12                # av strips 0..11 = fp8 pair region, 12..17 = bf16
GA = [(0, 8), (8, 6)]      # ACT-region spread groups (block ranges)
GD = [(0, 5), (5, 5), (10, 6)]  # DVE-region spread groups
DIRECT_A = (14, 4)         # blocks 14..17 (pairs p7, p8): direct in the tail
DIRECT_D = (16, 2)         # blocks 16,17: direct in the tail

# strips < dve_from(j) -> ACT exp; else DVE EXP16.  First blocks lean on
# ACT while DVE does the vv/kT setup copies.
DVE_FROM_LIST = [5, 5] + [4] * 16

# EXP16 poly: exp(16u)/16 ~ (g*c0 + u*(g*c1 + u*g*c2))^16, g = 16^(-1/16)
_G = 16.0 ** (-1.0 / 16.0)
EXPC = (0.9999280385484721 * _G, 1.000676421773311 * _G, 0.5251343712954386 * _G)

_CACHE = {}


def _register_exp16():
    """Register the EXP16 custom DVE op (idempotent)."""
    import concourse.dve_ops as do
    from concourse.dve_spec import Spec, Src0, C0, C1, C2, sq, lower, _has_src1
    from concourse.dve_uop import DveOpSpec

    for op in do.OPS:
        if op.name == "EXP16_ANT":
            return op
    u = Src0
    p = C2 + u * (C0 + u * C1)
    body = sq(sq(sq(sq(p))))

    def ref(in0, in1, c0, c1, c2):
        x = in0.astype(np.float32)
        pp = np.float32(c2) + x * (np.float32(c0) + x * np.float32(c1))
        r = pp * pp
        r = r * r
        r = r * r
        return r * r

    spec = Spec(body=body, reference=ref)
    row = max(do._SUB_OPCODE_FOR_NAME.values()) + 1
    assert row < 0x20, row
    shas = {}
    for ver in ("v3", "v4"):
        s = DveOpSpec(name="EXP16_ANT", opcode=row, uops=lower(spec, ver=ver),
                      rd1_en=_has_src1(spec))
        shas[ver] = s.sha(ver)
    op = do.DveOp("EXP16_ANT", spec, subdim=False, uops_sha=shas)
    do.OPS.append(op)
    do._SUB_OPCODE_FOR_NAME["EXP16_ANT"] = row
    do.CUSTOM_DVE_SPECS["EXP16_ANT"] = spec
    return op


def _build_phase1():
    import concourse.bacc as bacc
    import concourse.tile as tile
    from concourse import mybir

    exp16 = _register_exp16()

    f32 = mybir.dt.float32
    bf16 = mybir.dt.bfloat16
    fp8 = mybir.dt.float8e4
    AX = mybir.AxisListType
    AF = mybir.ActivationFunctionType
    ADD = mybir.AluOpType.add
    DR = mybir.MatmulPerfMode.DoubleRow

    nc = bacc.Bacc("TRN2", target_bir_lowering=False, debug=False)

    xBd_d = nc.dram_tensor("xBd", [KD, 2, N], fp8, kind="ExternalInput").ap()
    xA_aug_d = nc.dram_tensor("xA_aug", [CAUG, NCHUNK], bf16, kind="ExternalInput").ap()
    mv_d = nc.dram_tensor("mv", [CAUG, 2 * KD], bf16, kind="ExternalInput").ap()
    wk_d = nc.dram_tensor("wk", [CAUG, CI], bf16, kind="ExternalInput").ap()
    omap_d = nc.dram_tensor("omap", [128, CI], bf16, kind="ExternalInput").ap()
    avp_d = nc.dram_tensor("av_part", [CI, N], bf16, kind="ExternalOutput").ap()

    # region-wise group lookup: which group does block j belong to
    ga_of, gd_of = {}, {}
    for gi, (g0, ng) in enumerate(GA):
        for j in range(g0, g0 + ng):
            ga_of[j] = gi
    for gi, (g0, ng) in enumerate(GD):
        for j in range(g0, g0 + ng):
            gd_of[j] = gi

    with tile.TileContext(nc) as tc:
        with (
            tc.tile_pool(name="big", bufs=1) as big,
            tc.tile_pool(name="pers", bufs=1) as pers,
            tc.tile_pool(name="small", bufs=4) as small,
            tc.tile_pool(name="stats", bufs=8) as stats,
            tc.tile_pool(name="scp", bufs=2, space="PSUM") as scp,
            tc.tile_pool(name="avp", bufs=2, space="PSUM") as avp,
        ):
            # ---- warmup: ACT exp-table load + bias const before data ----
            warm = small.tile([128, 1], f32, tag="warm")
            nc.vector.memset(warm[:, :], 0.0)
            bln = small.tile([128, 1], f32, tag="bln", bufs=1)
            nc.vector.memset(bln[:, :], -LN16)
            warm2 = small.tile([128, 1], f32, tag="warm")
            nc.scalar.activation(warm2[:, :], warm[:, :], AF.Exp)

            # ---- input DMAs: mv + xA first chunk gate the vv sliver ----
            mv_sb = small.tile([CAUG, 2 * KD], bf16, tag="w")
            nc.sync.dma_start(mv_sb[:], mv_d[:])
            xA_sb = pers.tile([CAUG, NCHUNK], bf16, tag="xA")
            nc.sync.dma_start(xA_sb[:, 0:1152], xA_aug_d[:, 0:1152])
            xBd_sb = pers.tile([KD, 2, N], fp8, tag="xbd")
            nc.sync.dma_start(xBd_sb[:, :, 0:1536], xBd_d[:, :, 0:1536])
            nc.sync.dma_start(xA_sb[:, 1152:2304], xA_aug_d[:, 1152:2304])
            wk_sb = small.tile([CAUG, CI], bf16, tag="w")
            nc.gpsimd.dma_start(wk_sb[:], wk_d[:])
            omap_sb = small.tile([128, CI], bf16, tag="w")
            nc.gpsimd.dma_start(omap_sb[:], omap_d[:])
            for blk in range(1, NSTRIP):
                lo, hi = blk * 1536, (blk + 1) * 1536
                nc.sync.dma_start(xBd_sb[:, :, lo:hi], xBd_d[:, :, lo:hi])

            vv_sb = pers.tile([KD, 2, NCHUNK], fp8, tag="vv")
            kT_sb = pers.tile([128, NSUB * CI], bf16, tag="kT")
            av_acc = pers.tile([128, N], bf16, tag="avacc")

            # ---- vv sliver [*,0:128] for both halves: unblocks block 0 ----
            for h in range(2):
                pt = scp.tile([128, STRIP], f32, tag="sc")
                nc.tensor.matmul(
                    pt[0:KD, 0:128], mv_sb[:, h * KD:(h + 1) * KD],
                    xA_sb[:, 0:128], start=True, stop=True,
                )
                nc.vector.tensor_copy(vv_sb[:, h, 0:128], pt[0:KD, 0:128])

            # deferred setup work, one item per early exp-strip slot:
            # ('vv', h, base, width) or ('kt', half)
            setup_q = [("kt", 0), ("kt", 1)]
            for cbase, cw in ((128, 512), (640, 512), (1152, 512),
                              (1664, 512), (2176, 128)):
                for h in range(2):
                    setup_q.append(("vv", h, cbase, cw))

            def do_setup(item):
                if item[0] == "kt":
                    half = item[1]
                    pt = avp.tile([128, AVS], f32, tag="av")
                    for i, j in enumerate(range(9 * half, 9 * (half + 1))):
                        nc.tensor.matmul(
                            pt[:, i * CI:(i + 1) * CI],
                            xA_sb[:, j * 128:(j + 1) * 128],
                            wk_sb[:, :], start=True, stop=True,
                        )
                    nc.vector.tensor_copy(
                        kT_sb[:, half * 9 * CI:(half + 1) * 9 * CI],
                        pt[:, 0:9 * CI])
                else:
                    _, h, cbase, cw = item
                    pt = avp.tile([128, AVS], f32, tag="av")
                    nc.tensor.matmul(
                        pt[0:KD, 0:cw], mv_sb[:, h * KD:(h + 1) * KD],
                        xA_sb[:, cbase:cbase + cw], start=True, stop=True,
                    )
                    nc.vector.tensor_copy(
                        vv_sb[:, h, cbase:cbase + cw], pt[0:KD, 0:cw])

            # ---- main loop ----
            pair_tiles = [None] * (NSUB // 2)   # fp8 [128, 2, 6144]
            edve_tiles = [None] * NSUB          # bf16 [128, 3072]
            kts8_tiles = [None] * (NSUB // 2)   # fp8 [128, 2, CI] (x32 scale)
            kts16_tiles = [None] * NSUB         # bf16 [128, CI]   (x32 scale)
            av_queue = []   # ('A'|'D', group_index, strip)
            emitted = [0]
            slot_no = [0]

            def emit_av(region, gi, t):
                at = avp.tile([128, AVS], f32, tag="av")
                csl = slice(t * AVS, (t + 1) * AVS)
                if region == "A":
                    g0, ng = GA[gi]
                    npair = ng // 2
                    for p in range(npair):
                        pi = g0 // 2 + p
                        nc.tensor.matmul(
                            at[0:32, :],
                            kts8_tiles[pi][:, :, :],
                            pair_tiles[pi][:, :, csl],
                            start=p == 0, stop=p == npair - 1,
                            perf_mode=DR,
                        )
                    rows = 32
                else:
                    g0, ng = GD[gi]
                    for cg in range(ng):
                        pos = cg % 4
                        nc.tensor.matmul(
                            at[pos * 32:(pos + 1) * 32, :],
                            kts16_tiles[g0 + cg][:, :],
                            edve_tiles[g0 + cg][:, (t - RSPLIT) * AVS:
                                                (t - RSPLIT + 1) * AVS],
                            start=cg == pos, stop=cg + 4 >= ng,
                            tile_position=(0, pos * 32),
                        )
                    rows = min(ng, 4) * 32
                dst = av_acc[0:rows, csl]
                src = at[0:rows, :]
                if gi == 0:
                    nc.vector.tensor_copy(dst, src)
                else:
                    nc.vector.tensor_tensor(dst, dst, src, op=ADD)
                emitted[0] += 1

            total_slots = NSUB * NSTRIP

            for j in range(NSUB):
                if j % 2 == 0:
                    pair_tiles[j // 2] = big.tile(
                        [128, 2, RSPLIT * AVS], fp8, tag="epair", bufs=5,
                        name=f"epair{j // 2}")
                epair = pair_tiles[j // 2]
                edve = big.tile([128, N - RSPLIT * AVS], bf16, tag="edve", bufs=8)
                edve_tiles[j] = edve
                zp = stats.tile([128, 8], f32, tag="zp")
                dve_from = DVE_FROM_LIST[j]
                # DVE strips first (j>0): their exp runs back-to-back so the
                # score-psum recycles fast; spread work lands on ACT slots.
                order = (list(range(NSTRIP)) if j == 0 else
                         list(range(dve_from, NSTRIP)) + list(range(dve_from)))
                for s in order:
                    sc = scp.tile([128, STRIP], f32, tag="sc")
                    for t3 in range(STRIP // 512):
                        col = s * STRIP + t3 * 512
                        nc.tensor.matmul(
                            sc[:, t3 * 512:(t3 + 1) * 512],
                            vv_sb[:, :, j * 128:(j + 1) * 128],
                            xBd_sb[:, :, col:col + 512],
                            start=True, stop=True,
                            perf_mode=DR,
                        )
                    if s < 4:
                        e_dst = epair[:, j % 2, s * STRIP:(s + 1) * STRIP]
                    else:
                        e_dst = edve[:, (s - 4) * STRIP:(s - 3) * STRIP]
                    if s < dve_from:
                        nc.scalar.activation(
                            e_dst, sc[:, :], AF.Exp,
                            scale=16.0, bias=bln[:, :],
                            accum_out=zp[:, s:s + 1],
                        )
                    else:
                        nc.vector._custom_dve(
                            exp16, out=e_dst, in0=sc[:, :],
                            s0=EXPC[1], s1=EXPC[2], imm2=EXPC[0],
                        )
                        nc.vector.tensor_scalar(
                            e_dst, e_dst, 1.0, None,
                            op0=mybir.AluOpType.mult, op1=mybir.AluOpType.add,
                            accum_out=zp[:, s:s + 1],
                        )
                    slot_no[0] += 1
                    if s >= dve_from and j > 0:
                        continue  # keep DVE-strip slots free of spread work
                    if setup_q:
                        do_setup(setup_q.pop(0))
                    if av_queue:
                        emit_av(*av_queue.pop(0))
                    slots_left = (total_slots - slot_no[0]) * 2 // 3
                    if av_queue and len(av_queue) > slots_left:
                        emit_av(*av_queue.pop(0))
                z = stats.tile([128, 1], f32, tag="z")
                nc.vector.reduce_sum(z[:, :], zp[:, 0:NSTRIP], axis=AX.X)
                rinv = stats.tile([128, 1], f32, tag="rinv")
                nc.vector.reciprocal(rinv[:, :], z[:, :])
                if j % 2 == 0:
                    kts8_tiles[j // 2] = stats.tile([128, 2, CI], fp8, tag="k8",
                                                    name=f"k8_{j // 2}")
                nc.vector.tensor_scalar(
                    kts8_tiles[j // 2][:, j % 2, :],
                    kT_sb[:, j * CI:(j + 1) * CI],
                    rinv[:, :], 32.0,
                    op0=mybir.AluOpType.mult, op1=mybir.AluOpType.mult,
                )
                kts16 = stats.tile([128, CI], bf16, tag="k16")
                kts16_tiles[j] = kts16
                nc.vector.tensor_scalar(
                    kts16[:, :], kT_sb[:, j * CI:(j + 1) * CI],
                    rinv[:, :], 32.0,
                    op0=mybir.AluOpType.mult, op1=mybir.AluOpType.mult,
                )
                gi = ga_of.get(j)
                if gi is not None and j == GA[gi][0] + GA[gi][1] - 1:
                    av_queue.extend(("A", gi, t) for t in range(RSPLIT))
                gi = gd_of.get(j)
                if gi is not None and j == GD[gi][0] + GD[gi][1] - 1:
                    av_queue.extend(("D", gi, t) for t in range(RSPLIT, NAVS))

            # ---- tail: leftover spread, then fused reduce + direct pair ----
            for it in av_queue:
                emit_av(*it)
            av_out = big.tile([CI, N], bf16, tag="avout", bufs=1)
            for t in range(NAVS):
                pool = scp if t % 2 == 0 else avp
                tag = "sc" if t % 2 == 0 else "av"
                rt = pool.tile([128, AVS], f32, tag=tag)
                csl = slice(t * AVS, (t + 1) * AVS)
                if t < RSPLIT:
                    # ACT region: av_acc rows 0:32 hold the full spread sum;
                    # omap[0:32] is the 32x32 identity.
                    nc.tensor.matmul(
                        rt[0:CI, :], omap_sb[0:32, :], av_acc[0:32, csl],
                        start=True, stop=False,
                    )
                    da0, dan = DIRECT_A
                    for p in range(dan // 2):
                        pi = da0 // 2 + p
                        nc.tensor.matmul(
                            rt[0:CI, :],
                            kts8_tiles[pi][:, :, :],
                            pair_tiles[pi][:, :, csl],
                            start=False, stop=(p == dan // 2 - 1),
                            perf_mode=DR,
                        )
                else:
                    nc.tensor.matmul(
                        rt[0:CI, :], omap_sb[:, :], av_acc[:, csl],
                        start=True, stop=False,
                    )
                    dd0, ddn = DIRECT_D
                    for d in range(ddn):
                        nc.tensor.matmul(
                            rt[0:CI, :],
                            kts16_tiles[dd0 + d][:, :],
                            edve_tiles[dd0 + d][:, (t - RSPLIT) * AVS:
                                                (t - RSPLIT + 1) * AVS],
                            start=False, stop=(d == ddn - 1),
                        )
                dst = av_out[:, csl]
                if t % 2 == 0:
                    nc.scalar.activation(dst, rt[0:CI, :], AF.Copy,
                                         bias=0.0, scale=1.0 / 32.0)
                else:
                    nc.vector.tensor_scalar_mul(dst, rt[0:CI, :], 1.0 / 32.0)
                if t % 3 == 2 or t == NAVS - 1:
                    lo = (t - (t % 3)) * AVS
                    nc.sync.dma_start(
                        avp_d[:, lo:(t + 1) * AVS], av_out[:, lo:(t + 1) * AVS]
                    )

    nc.compile()
    return nc


def _build_phase2():
    import concourse.bacc as bacc
    import concourse.tile as tile
    from concourse import mybir

    f32 = mybir.dt.float32
    bf16 = mybir.dt.bfloat16
    AF = mybir.ActivationFunctionType
    MQ = N // 4   # 2304 output columns per core
    K2 = CI + 1 + C  # 97 contraction rows: WfinT | cfin | I (residual)

    nc = bacc.Bacc("TRN2", target_bir_lowering=False, debug=False)

    avs_d = nc.dram_tensor("avs", [K2, C + MQ], bf16, kind="ExternalInput").ap()
    out_d = nc.dram_tensor("outc", [C, MQ], f32, kind="ExternalOutput").ap()

    with tile.TileContext(nc) as tc:
        with (
            tc.tile_pool(name="sb", bufs=1) as sb,
            tc.tile_pool(name="ps", bufs=4, space="PSUM") as ps,
        ):
            warm = sb.tile([128, 1], f32, tag="warm")
            nc.vector.memset(warm[:, :], 0.0)
            warm2 = sb.tile([128, 1], f32, tag="warm2")
            nc.scalar.activation(warm2[:, :], warm[:, :], AF.Relu)

            av_aug = sb.tile([K2, C + MQ], bf16, tag="avaug")
            nc.sync.dma_start(av_aug[:, 0:C + 512], avs_d[:, 0:C + 512])
            nc.scalar.dma_start(av_aug[:, C + 512:C + 1024], avs_d[:, C + 512:C + 1024])
            nc.sync.dma_start(av_aug[:, C + 1024:C + 1536], avs_d[:, C + 1024:C + 1536])
            nc.scalar.dma_start(av_aug[:, C + 1536:C + 2048], avs_d[:, C + 1536:C + 2048])
            nc.sync.dma_start(av_aug[:, C + 2048:C + MQ], avs_d[:, C + 2048:C + MQ])
            o_sb = sb.tile([C, MQ], f32, tag="o")

            nstr = (MQ + 511) // 512
            for s in range(nstr):
                sw = min(512, MQ - s * 512)
                sl = slice(s * 512, s * 512 + sw)
                op = ps.tile([128, 512], f32, tag="rp")
                nc.tensor.matmul(
                    op[0:C, 0:sw], av_aug[:, 0:C],
                    av_aug[:, C + s * 512:C + s * 512 + sw],
                    start=True, stop=True,
                )
                nc.scalar.activation(o_sb[:, sl], op[0:C, 0:sw], AF.Relu)
                nc.sync.dma_start(out_d[:, sl], o_sb[:, sl])

    nc.compile()
    return nc


def _get_programs():
    if "p1" not in _CACHE:
        _CACHE["p1"] = _build_phase1()
        _CACHE["p2"] = _build_phase2()
    return _CACHE["p1"], _CACHE["p2"]


def _ones_map(dtype):
    m = np.zeros((128, CI), dtype)
    for g in range(4):
        m[g * 32 + np.arange(CI), np.arange(CI)] = 1
    return m


def kernel(xA, xB, Wk, bk, Wv, bv, Wq, bq, Wg,
           g1_gamma, g1_beta, g1_mean, g1_var,
           Wo, bo, g2_gamma, g2_beta, g2_mean, g2_var):
    from concourse.bass_utils import run_bass_kernel_spmd

    p1, p2 = _get_programs()

    xA = np.asarray(xA, np.float32).reshape(B, C, N)
    xB = np.asarray(xB, np.float32).reshape(B, C, N)

    # ---- host-side weight folding (tiny) ----
    s1 = np.asarray(g1_gamma) / np.sqrt(np.asarray(g1_var) + EPS)
    Wg_f = s1[:, None] * np.asarray(Wg)
    c1 = np.asarray(g1_beta) - s1 * np.asarray(g1_mean)
    s2 = np.asarray(g2_gamma) / np.sqrt(np.asarray(g2_var) + EPS)
    Wo_f = s2[:, None] * np.asarray(Wo)
    c2 = s2 * (np.asarray(bo) - np.asarray(g2_mean)) + np.asarray(g2_beta)
    Wfin = (Wo_f @ Wg_f).astype(np.float32)          # [C, CI]
    cfin = (Wo_f @ c1 + c2).astype(np.float32)       # [C]

    # fold both score projections into M = Wv_aug^T Wq_aug / 2 (so that
    # (M^T xA)^T (xB/8) = s/16), padded to 66 columns for the dual layout
    Wv_aug = np.concatenate([np.asarray(Wv), np.asarray(bv)[:, None]], 1)
    Wq_aug = np.concatenate([np.asarray(Wq), np.asarray(bq)[:, None]], 1)
    M = (Wv_aug.T @ Wq_aug / 2.0).astype(np.float32)         # [65, 65]
    mv = np.concatenate([M, np.zeros((CAUG, 1), np.float32)], 1).astype(BF16)
    wk_aug = np.concatenate([np.asarray(Wk).T, np.asarray(bk)[None, :]], 0).astype(BF16)
    omap16 = _ones_map(BF16)

    ones_n = np.ones((1, N), np.float32)

    # xBd: [66, N]/8 -> fp8 dual [33, 2, N] (pure layout prep)
    xBd = []
    for b in range(B):
        xb66 = np.concatenate([xB[b] / 8.0, ones_n / 8.0,
                               np.zeros((1, N), np.float32)], 0)
        xBd.append(np.ascontiguousarray(
            xb66.reshape(2, KD, N).transpose(1, 0, 2)).astype(FP8))

    # ---- phase 1: per-core (batch, key-row chunk) partial attention ----
    in_maps1 = []
    for core in range(NCORES):
        b, chunk = divmod(core, 4)
        sl = slice(chunk * NCHUNK, (chunk + 1) * NCHUNK)
        in_maps1.append({
            "xBd": xBd[b],
            "xA_aug": np.concatenate([xA[b][:, sl], ones_n[:, sl]], 0).astype(BF16),
            "mv": mv, "wk": wk_aug,
            "omap": omap16,
        })
    res1 = run_bass_kernel_spmd(p1, in_maps1, list(range(NCORES)))
    av_parts = [res1.results[i]["av_part"].astype(np.float32) for i in range(NCORES)]

    # ---- phase 2: per-core (batch, query chunk) epilogue ----
    MQ = N // 4
    av_sum = [sum(av_parts[b * 4 + i] for i in range(4)) for b in range(B)]
    wblk = np.concatenate([Wfin.T, cfin[None, :], np.eye(C, dtype=np.float32)], 0)
    ones_mq = np.ones((1, MQ), np.float32)
    in_maps2 = []
    for core in range(NCORES):
        b, mq = divmod(core, 4)
        msl = slice(mq * MQ, (mq + 1) * MQ)
        rhs = np.concatenate([av_sum[b][:, msl], ones_mq, xB[b][:, msl]], 0)
        in_maps2.append({
            "avs": np.concatenate([wblk, rhs], 1).astype(BF16),
        })
    res2 = run_bass_kernel_spmd(p2, in_maps2, list(range(NCORES)))

    out = np.zeros((B, C, N), np.float32)
    for core in range(NCORES):
        b, mq = divmod(core, 4)
        out[b][:, mq * MQ:(mq + 1) * MQ] = res2.results[core]["outc"]
    return out.reshape(B, C, H, W)


# revision 12
# speedup vs baseline: 1.1124x; 1.0313x over previous
"""Trainium2 Bass kernel for nn_MFA_87067577025371.

Architecture (B=2, C=64, Ci=32, H=W=96, N=9216):
  k,v = 1x1conv(xA); q = 1x1conv(xB)
  A   = softmax(v^T q, axis=2)            # [B, N, N], softmax over query dim m
  av  = k @ A                             # [B, Ci, N]
  out = relu(BN2(Wo @ BN1(Wg @ av)) + xB)

Sharding: (batch, key-row chunk) across 8 cores — each core owns 2304 rows
of the score matrix for one batch, computes E = exp(s)/16 for all m,
row-sums Z' = Z/16, scales kT by 32/Z' and accumulates its partial
av*32 = kts^T E (the uniform 1/16 E scale and x32 kts scale cancel in
softmax normalization; the tail copies rescale by 1/32).  A second tiny
launch applies the host-folded epilogue per (batch, query chunk).

Speed structure (v4):
  * both score projections fold on host into M = Wv_aug^T Wq_aug/2, so
    s/16 = vv^T xBd with vv = M^T xA (tiny on-device projection, fp8) and
    xBd = xB/8 quantized to fp8 on host (layout prep only).  Score
    matmuls run fp8e4 DoubleRow at 0.5 PE-cycles/column.
  * exp splits across engines: ~4.25 strips/block on ACT via
    activation(Exp, scale=16, bias=-ln16, accum_out=Zpartial), the rest
    on DVE via a custom op EXP16 = (c0+u(c1+u c2))^16 with the 1/16
    folded into the coefficients.  End-to-end ~1.7e-3 rel err.
  * E column region [0:6144] is stored fp8 in block-PAIR tiles
    [128,2,6144] so av matmuls for that region run DoubleRow pairs (2
    blocks per instruction); region [6144:9216] stays bf16 (DVE needs
    2-byte dtype for its fast row-sum pass on the strips it exp'd).
  * av spreads through av_acc with region-wise groups (ACT region: two
    groups of 8, all-pair emits; DVE region: 5/5/6) and a direct block
    pair (16,17) folded into the tail reduce.
"""

import os
import sys

import numpy as np

for _p in ("/opt/trn_rl_repo", "/root/.axon_site/_ro/trn_rl_repo"):
    if os.path.isdir(_p) and _p not in sys.path:
        sys.path.insert(0, _p)

import ml_dtypes  # noqa: E402

BF16 = ml_dtypes.bfloat16
FP8 = ml_dtypes.float8_e4m3fn

# ---- problem constants (hardcoded per contract) ----
B, C, CI, H, W = 2, 64, 32, 96, 96
N = H * W                  # 9216
NCORES = 8
NCHUNK = N // 4            # 2304 key rows per core
NSUB = NCHUNK // 128       # 18 blocks of 128 rows
STRIP = 1536               # exp strip (3 PSUM banks)
NSTRIP = N // STRIP        # 6
AVS = 512                  # av matmul strip
NAVS = N // AVS            # 18
CAUG = C + 1               # 65 (bias row folded in)
KD = 33                    # dual-layout contraction rows (65 ch + pad)/2
EPS = 1e-5
LN16 = float(np.log(16.0))

RSPLIT = 12                # av strips 0..11 = fp8 pair region, 12..17 = bf16
GA = [(0, 8), (8, 6)]      # ACT-region spread groups (block ranges)
GD = [(0, 5), (5, 5), (10, 6)]  # DVE-region spread groups
DIRECT_A = (14, 4)         # blocks 14..17 (pairs p7, p8): direct in the tail
DIRECT_D = (16, 2)         # blocks 16,17: direct in the tail

# strips < dve_from(j) -> ACT exp; else DVE EXP16.  First blocks lean on
# ACT while DVE does the vv/kT setup copies.
DVE_FROM_LIST = [5, 5] + [4] * 16

# EXP16 poly: exp(16u)/16 ~ (g*c0 + u*(g*c1 + u*g*c2))^16, g = 16^(-1/16)
_G = 16.0 ** (-1.0 / 16.0)
EXPC = (0.9999280385484721 * _G, 1.000676421773311 * _G, 0.5251343712954386 * _G)

_CACHE = {}


def _register_exp16():
    """Register the EXP16 custom DVE op (idempotent)."""
    import concourse.dve_ops as do
    from concourse.dve_spec import Spec, Src0, C0, C1, C2, sq, lower, _has_src1
    from concourse.dve_uop import DveOpSpec

    for op in do.OPS:
        if op.name == "EXP16_ANT":
            return op
    u = Src0
    p = C2 + u * (C0 + u * C1)
    body = sq(sq(sq(sq(p))))

    def ref(in0, in1, c0, c1, c2):
        x = in0.astype(np.float32)
        pp = np.float32(c2) + x * (np.float32(c0) + x * np.float32(c1))
        r = pp * pp
        r = r * r
        r = r * r
        return r * r

    spec = Spec(body=body, reference=ref)
    row = max(do._SUB_OPCODE_FOR_NAME.values()) + 1
    assert row < 0x20, row
    shas = {}
    for ver in ("v3", "v4"):
        s = DveOpSpec(name="EXP16_ANT", opcode=row, uops=lower(spec, ver=ver),
                      rd1_en=_has_src1(spec))
        shas[ver] = s.sha(ver)
    op = do.DveOp("EXP16_ANT", spec, subdim=False, uops_sha=shas)
    do.OPS.append(op)
    do._SUB_OPCODE_FOR_NAME["EXP16_ANT"] = row
    do.CUSTOM_DVE_SPECS["EXP16_ANT"] = spec
    return op


def _build_phase1():
    import concourse.bacc as bacc
    import concourse.tile as tile
    from concourse import mybir

    exp16 = _register_exp16()

    f32 = mybir.dt.float32
    bf16 = mybir.dt.bfloat16
    fp8 = mybir.dt.float8e4
    AX = mybir.AxisListType
    AF = mybir.ActivationFunctionType
    ADD = mybir.AluOpType.add
    DR = mybir.MatmulPerfMode.DoubleRow

    nc = bacc.Bacc("TRN2", target_bir_lowering=False, debug=False)

    xBd_d = nc.dram_tensor("xBd", [KD, 2, N], fp8, kind="ExternalInput").ap()
    xA_aug_d = nc.dram_tensor("xA_aug", [CAUG, NCHUNK], bf16, kind="ExternalInput").ap()
    mv_d = nc.dram_tensor("mv", [CAUG, 2 * KD], bf16, kind="ExternalInput").ap()
    wk_d = nc.dram_tensor("wk", [CAUG, CI], bf16, kind="ExternalInput").ap()
    omap_d = nc.dram_tensor("omap", [128, CI], bf16, kind="ExternalInput").ap()
    avp_d = nc.dram_tensor("av_part", [CI, N], bf16, kind="ExternalOutput").ap()

    # region-wise group lookup: which group does block j belong to
    ga_of, gd_of = {}, {}
    for gi, (g0, ng) in enumerate(GA):
        for j in range(g0, g0 + ng):
            ga_of[j] = gi
    for gi, (g0, ng) in enumerate(GD):
        for j in range(g0, g0 + ng):
            gd_of[j] = gi

    with tile.TileContext(nc) as tc:
        with (
            tc.tile_pool(name="big", bufs=1) as big,
            tc.tile_pool(name="pers", bufs=1) as pers,
            tc.tile_pool(name="small", bufs=4) as small,
            tc.tile_pool(name="stats", bufs=8) as stats,
            tc.tile_pool(name="scp", bufs=2, space="PSUM") as scp,
            tc.tile_pool(name="avp", bufs=2, space="PSUM") as avp,
        ):
            # ---- warmup: ACT exp-table load + bias const before data ----
            warm = small.tile([128, 1], f32, tag="warm")
            nc.vector.memset(warm[:, :], 0.0)
            bln = small.tile([128, 1], f32, tag="bln", bufs=1)
            nc.vector.memset(bln[:, :], -LN16)
            warm2 = small.tile([128, 1], f32, tag="warm")
            nc.scalar.activation(warm2[:, :], warm[:, :], AF.Exp)

            # ---- input DMAs: mv + xA first chunk gate the vv sliver ----
            mv_sb = small.tile([CAUG, 2 * KD], bf16, tag="w")
            nc.sync.dma_start(mv_sb[:], mv_d[:])
            xA_sb = pers.tile([CAUG, NCHUNK], bf16, tag="xA")
            nc.sync.dma_start(xA_sb[:, 0:1152], xA_aug_d[:, 0:1152])
            xBd_sb = pers.tile([KD, 2, N], fp8, tag="xbd")
            nc.sync.dma_start(xBd_sb[:, :, 0:1536], xBd_d[:, :, 0:1536])
            nc.sync.dma_start(xA_sb[:, 1152:2304], xA_aug_d[:, 1152:2304])
            wk_sb = small.tile([CAUG, CI], bf16, tag="w")
            nc.gpsimd.dma_start(wk_sb[:], wk_d[:])
            omap_sb = small.tile([128, CI], bf16, tag="w")
            nc.gpsimd.dma_start(omap_sb[:], omap_d[:])
            for blk in range(1, NSTRIP):
                lo, hi = blk * 1536, (blk + 1) * 1536
                nc.sync.dma_start(xBd_sb[:, :, lo:hi], xBd_d[:, :, lo:hi])

            vv_sb = pers.tile([KD, 2, NCHUNK], fp8, tag="vv")
            kT_sb = pers.tile([128, NSUB * CI], bf16, tag="kT")
            av_acc = pers.tile([128, N], bf16, tag="avacc")

            # ---- vv sliver [*,0:128] for both halves: unblocks block 0 ----
            for h in range(2):
                pt = scp.tile([128, STRIP], f32, tag="sc")
                nc.tensor.matmul(
                    pt[0:KD, 0:128], mv_sb[:, h * KD:(h + 1) * KD],
                    xA_sb[:, 0:128], start=True, stop=True,
                )
                nc.vector.tensor_copy(vv_sb[:, h, 0:128], pt[0:KD, 0:128])

            # deferred setup work, one item per early exp-strip slot:
            # ('vv', h, base, width) or ('kt', half)
            setup_q = [("kt", 0), ("kt", 1)]
            for cbase, cw in ((128, 512), (640, 512), (1152, 512),
                              (1664, 512), (2176, 128)):
                for h in range(2):
                    setup_q.append(("vv", h, cbase, cw))

            def do_setup(item):
                if item[0] == "kt":
                    half = item[1]
                    pt = avp.tile([128, AVS], f32, tag="av")
                    for i, j in enumerate(range(9 * half, 9 * (half + 1))):
                        nc.tensor.matmul(
                            pt[:, i * CI:(i + 1) * CI],
                            xA_sb[:, j * 128:(j + 1) * 128],
                            wk_sb[:, :], start=True, stop=True,
                        )
                    nc.vector.tensor_copy(
                        kT_sb[:, half * 9 * CI:(half + 1) * 9 * CI],
                        pt[:, 0:9 * CI])
                else:
                    _, h, cbase, cw = item
                    pt = avp.tile([128, AVS], f32, tag="av")
                    nc.tensor.matmul(
                        pt[0:KD, 0:cw], mv_sb[:, h * KD:(h + 1) * KD],
                        xA_sb[:, cbase:cbase + cw], start=True, stop=True,
                    )
                    nc.vector.tensor_copy(
                        vv_sb[:, h, cbase:cbase + cw], pt[0:KD, 0:cw])

            # ---- main loop ----
            pair_tiles = [None] * (NSUB // 2)   # fp8 [128, 2, 6144]
            edve_tiles = [None] * NSUB          # bf16 [128, 3072]
            kts8_tiles = [None] * (NSUB // 2)   # fp8 [128, 2, CI] (x32 scale)
            kts16_tiles = [None] * NSUB         # bf16 [128, CI]   (x32 scale)
            av_queue = []   # ('A'|'D', group_index, strip)
            emitted = [0]
            slot_no = [0]

            def emit_av(region, gi, t):
                at = avp.tile([128, AVS], f32, tag="av")
                csl = slice(t * AVS, (t + 1) * AVS)
                if region == "A":
                    g0, ng = GA[gi]
                    npair = ng // 2
                    for p in range(npair):
                        pi = g0 // 2 + p
                        nc.tensor.matmul(
                            at[0:32, :],
                            kts8_tiles[pi][:, :, :],
                            pair_tiles[pi][:, :, csl],
                            start=p == 0, stop=p == npair - 1,
                            perf_mode=DR,
                        )
                    rows = 32
                else:
                    g0, ng = GD[gi]
                    for cg in range(ng):
                        pos = cg % 4
                        nc.tensor.matmul(
                            at[pos * 32:(pos + 1) * 32, :],
                            kts16_tiles[g0 + cg][:, :],
                            edve_tiles[g0 + cg][:, (t - RSPLIT) * AVS:
                                                (t - RSPLIT + 1) * AVS],
                            start=cg == pos, stop=cg + 4 >= ng,
                            tile_position=(0, pos * 32),
                        )
                    rows = min(ng, 4) * 32
                dst = av_acc[0:rows, csl]
                src = at[0:rows, :]
                if gi == 0:
                    nc.vector.tensor_copy(dst, src)
                else:
                    nc.vector.tensor_tensor(dst, dst, src, op=ADD)
                emitted[0] += 1

            total_slots = NSUB * NSTRIP

            for j in range(NSUB):
                if j % 2 == 0:
                    pair_tiles[j // 2] = big.tile(
                        [128, 2, RSPLIT * AVS], fp8, tag="epair", bufs=5,
                        name=f"epair{j // 2}")
                epair = pair_tiles[j // 2]
                edve = big.tile([128, N - RSPLIT * AVS], bf16, tag="edve", bufs=8)
                edve_tiles[j] = edve
                zp = stats.tile([128, 8], f32, tag="zp")
                dve_from = DVE_FROM_LIST[j]
                # interleave DVE strips between ACT strips so the next ACT
                # strip is never more than 2 psum allocations behind.
                if j == 0:
                    order = list(range(NSTRIP))
                elif dve_from == 4:
                    order = [4, 0, 5, 1, 2, 3]
                else:
                    order = [5, 0, 1, 2, 3, 4]
                for s in order:
                    sc = scp.tile([128, STRIP], f32, tag="sc")
                    for t3 in range(STRIP // 512):
                        col = s * STRIP + t3 * 512
                        nc.tensor.matmul(
                            sc[:, t3 * 512:(t3 + 1) * 512],
                            vv_sb[:, :, j * 128:(j + 1) * 128],
                            xBd_sb[:, :, col:col + 512],
                            start=True, stop=True,
                            perf_mode=DR,
                        )
                    if s < 4:
                        e_dst = epair[:, j % 2, s * STRIP:(s + 1) * STRIP]
                    else:
                        e_dst = edve[:, (s - 4) * STRIP:(s - 3) * STRIP]
                    if s < dve_from:
                        nc.scalar.activation(
                            e_dst, sc[:, :], AF.Exp,
                            scale=16.0, bias=bln[:, :],
                            accum_out=zp[:, s:s + 1],
                        )
                    else:
                        nc.vector._custom_dve(
                            exp16, out=e_dst, in0=sc[:, :],
                            s0=EXPC[1], s1=EXPC[2], imm2=EXPC[0],
                        )
                        nc.vector.tensor_scalar(
                            e_dst, e_dst, 1.0, None,
                            op0=mybir.AluOpType.mult, op1=mybir.AluOpType.add,
                            accum_out=zp[:, s:s + 1],
                        )
                    slot_no[0] += 1
                    if s >= dve_from and j > 0:
                        continue  # keep DVE-strip slots free of spread work
                    if setup_q:
                        do_setup(setup_q.pop(0))
                    if av_queue:
                        emit_av(*av_queue.pop(0))
                    slots_left = (total_slots - slot_no[0]) * 2 // 3
                    if av_queue and len(av_queue) > slots_left:
                        emit_av(*av_queue.pop(0))
                z = stats.tile([128, 1], f32, tag="z")
                nc.vector.reduce_sum(z[:, :], zp[:, 0:NSTRIP], axis=AX.X)
                rinv = stats.tile([128, 1], f32, tag="rinv")
                nc.vector.reciprocal(rinv[:, :], z[:, :])
                if j % 2 == 0:
                    kts8_tiles[j // 2] = stats.tile([128, 2, CI], fp8, tag="k8",
                                                    name=f"k8_{j // 2}")
                nc.vector.tensor_scalar(
                    kts8_tiles[j // 2][:, j % 2, :],
                    kT_sb[:, j * CI:(j + 1) * CI],
                    rinv[:, :], 32.0,
                    op0=mybir.AluOpType.mult, op1=mybir.AluOpType.mult,
                )
                kts16 = stats.tile([128, CI], bf16, tag="k16")
                kts16_tiles[j] = kts16
                nc.vector.tensor_scalar(
                    kts16[:, :], kT_sb[:, j * CI:(j + 1) * CI],
                    rinv[:, :], 32.0,
                    op0=mybir.AluOpType.mult, op1=mybir.AluOpType.mult,
                )
                gi = ga_of.get(j)
                if gi is not None and j == GA[gi][0] + GA[gi][1] - 1:
                    av_queue.extend(("A", gi, t) for t in range(RSPLIT))
                gi = gd_of.get(j)
                if gi is not None and j == GD[gi][0] + GD[gi][1] - 1:
                    av_queue.extend(("D", gi, t) for t in range(RSPLIT, NAVS))

            # ---- tail: leftover spread, then fused reduce + direct pair ----
            for it in av_queue:
                emit_av(*it)
            av_out = big.tile([CI, N], bf16, tag="avout", bufs=1)
            for t in range(NAVS):
                pool = scp if t % 2 == 0 else avp
                tag = "sc" if t % 2 == 0 else "av"
                rt = pool.tile([128, AVS], f32, tag=tag)
                csl = slice(t * AVS, (t + 1) * AVS)
                if t < RSPLIT:
                    # ACT region: av_acc rows 0:32 hold the full spread sum;
                    # omap[0:32] is the 32x32 identity.
                    nc.tensor.matmul(
                        rt[0:CI, :], omap_sb[0:32, :], av_acc[0:32, csl],
                        start=True, stop=False,
                    )
                    da0, dan = DIRECT_A
                    for p in range(dan // 2):
                        pi = da0 // 2 + p
                        nc.tensor.matmul(
                            rt[0:CI, :],
                            kts8_tiles[pi][:, :, :],
                            pair_tiles[pi][:, :, csl],
                            start=False, stop=(p == dan // 2 - 1),
                            perf_mode=DR,
                        )
                else:
                    nc.tensor.matmul(
                        rt[0:CI, :], omap_sb[:, :], av_acc[:, csl],
                        start=True, stop=False,
                    )
                    dd0, ddn = DIRECT_D
                    for d in range(ddn):
                        nc.tensor.matmul(
                            rt[0:CI, :],
                            kts16_tiles[dd0 + d][:, :],
                            edve_tiles[dd0 + d][:, (t - RSPLIT) * AVS:
                                                (t - RSPLIT + 1) * AVS],
                            start=False, stop=(d == ddn - 1),
                        )
                dst = av_out[:, csl]
                if t % 2 == 0:
                    nc.scalar.activation(dst, rt[0:CI, :], AF.Copy,
                                         bias=0.0, scale=1.0 / 32.0)
                else:
                    nc.vector.tensor_scalar_mul(dst, rt[0:CI, :], 1.0 / 32.0)
                if t % 3 == 2 or t == NAVS - 1:
                    lo = (t - (t % 3)) * AVS
                    nc.sync.dma_start(
                        avp_d[:, lo:(t + 1) * AVS], av_out[:, lo:(t + 1) * AVS]
                    )

    nc.compile()
    return nc


def _build_phase2():
    import concourse.bacc as bacc
    import concourse.tile as tile
    from concourse import mybir

    f32 = mybir.dt.float32
    bf16 = mybir.dt.bfloat16
    AF = mybir.ActivationFunctionType
    MQ = N // 4   # 2304 output columns per core
    K2 = CI + 1 + C  # 97 contraction rows: WfinT | cfin | I (residual)

    nc = bacc.Bacc("TRN2", target_bir_lowering=False, debug=False)

    avs_d = nc.dram_tensor("avs", [K2, C + MQ], bf16, kind="ExternalInput").ap()
    out_d = nc.dram_tensor("outc", [C, MQ], f32, kind="ExternalOutput").ap()

    with tile.TileContext(nc) as tc:
        with (
            tc.tile_pool(name="sb", bufs=1) as sb,
            tc.tile_pool(name="ps", bufs=4, space="PSUM") as ps,
        ):
            warm = sb.tile([128, 1], f32, tag="warm")
            nc.vector.memset(warm[:, :], 0.0)
            warm2 = sb.tile([128, 1], f32, tag="warm2")
            nc.scalar.activation(warm2[:, :], warm[:, :], AF.Relu)

            av_aug = sb.tile([K2, C + MQ], bf16, tag="avaug")
            nc.sync.dma_start(av_aug[:, 0:C + 512], avs_d[:, 0:C + 512])
            nc.scalar.dma_start(av_aug[:, C + 512:C + 1408], avs_d[:, C + 512:C + 1408])
            nc.sync.dma_start(av_aug[:, C + 1408:C + MQ], avs_d[:, C + 1408:C + MQ])
            o_sb = sb.tile([C, MQ], f32, tag="o")

            nstr = (MQ + 511) // 512
            for s in range(nstr):
                sw = min(512, MQ - s * 512)
                sl = slice(s * 512, s * 512 + sw)
                op = ps.tile([128, 512], f32, tag="rp")
                nc.tensor.matmul(
                    op[0:C, 0:sw], av_aug[:, 0:C],
                    av_aug[:, C + s * 512:C + s * 512 + sw],
                    start=True, stop=True,
                )
                nc.scalar.activation(o_sb[:, sl], op[0:C, 0:sw], AF.Relu)
                nc.sync.dma_start(out_d[:, sl], o_sb[:, sl])

    nc.compile()
    return nc


def _get_programs():
    if "p1" not in _CACHE:
        _CACHE["p1"] = _build_phase1()
        _CACHE["p2"] = _build_phase2()
    return _CACHE["p1"], _CACHE["p2"]


def _ones_map(dtype):
    m = np.zeros((128, CI), dtype)
    for g in range(4):
        m[g * 32 + np.arange(CI), np.arange(CI)] = 1
    return m


def kernel(xA, xB, Wk, bk, Wv, bv, Wq, bq, Wg,
           g1_gamma, g1_beta, g1_mean, g1_var,
           Wo, bo, g2_gamma, g2_beta, g2_mean, g2_var):
    from concourse.bass_utils import run_bass_kernel_spmd

    p1, p2 = _get_programs()

    xA = np.asarray(xA, np.float32).reshape(B, C, N)
    xB = np.asarray(xB, np.float32).reshape(B, C, N)

    # ---- host-side weight folding (tiny) ----
    s1 = np.asarray(g1_gamma) / np.sqrt(np.asarray(g1_var) + EPS)
    Wg_f = s1[:, None] * np.asarray(Wg)
    c1 = np.asarray(g1_beta) - s1 * np.asarray(g1_mean)
    s2 = np.asarray(g2_gamma) / np.sqrt(np.asarray(g2_var) + EPS)
    Wo_f = s2[:, None] * np.asarray(Wo)
    c2 = s2 * (np.asarray(bo) - np.asarray(g2_mean)) + np.asarray(g2_beta)
    Wfin = (Wo_f @ Wg_f).astype(np.float32)          # [C, CI]
    cfin = (Wo_f @ c1 + c2).astype(np.float32)       # [C]

    # fold both score projections into M = Wv_aug^T Wq_aug / 2 (so that
    # (M^T xA)^T (xB/8) = s/16), padded to 66 columns for the dual layout
    Wv_aug = np.concatenate([np.asarray(Wv), np.asarray(bv)[:, None]], 1)
    Wq_aug = np.concatenate([np.asarray(Wq), np.asarray(bq)[:, None]], 1)
    M = (Wv_aug.T @ Wq_aug / 2.0).astype(np.float32)         # [65, 65]
    mv = np.concatenate([M, np.zeros((CAUG, 1), np.float32)], 1).astype(BF16)
    wk_aug = np.concatenate([np.asarray(Wk).T, np.asarray(bk)[None, :]], 0).astype(BF16)
    omap16 = _ones_map(BF16)

    ones_n = np.ones((1, N), np.float32)

    # xBd: [66, N]/8 -> fp8 dual [33, 2, N] (pure layout prep)
    xBd = []
    for b in range(B):
        xb66 = np.concatenate([xB[b] / 8.0, ones_n / 8.0,
                               np.zeros((1, N), np.float32)], 0)
        xBd.append(np.ascontiguousarray(
            xb66.reshape(2, KD, N).transpose(1, 0, 2)).astype(FP8))

    # ---- phase 1: per-core (batch, key-row chunk) partial attention ----
    in_maps1 = []
    for core in range(NCORES):
        b, chunk = divmod(core, 4)
        sl = slice(chunk * NCHUNK, (chunk + 1) * NCHUNK)
        in_maps1.append({
            "xBd": xBd[b],
            "xA_aug": np.concatenate([xA[b][:, sl], ones_n[:, sl]], 0).astype(BF16),
            "mv": mv, "wk": wk_aug,
            "omap": omap16,
        })
    res1 = run_bass_kernel_spmd(p1, in_maps1, list(range(NCORES)))
    av_parts = [res1.results[i]["av_part"].astype(np.float32) for i in range(NCORES)]

    # ---- phase 2: per-core (batch, query chunk) epilogue ----
    MQ = N // 4
    av_sum = [sum(av_parts[b * 4 + i] for i in range(4)) for b in range(B)]
    wblk = np.concatenate([Wfin.T, cfin[None, :], np.eye(C, dtype=np.float32)], 0)
    ones_mq = np.ones((1, MQ), np.float32)
    in_maps2 = []
    for core in range(NCORES):
        b, mq = divmod(core, 4)
        msl = slice(mq * MQ, (mq + 1) * MQ)
        rhs = np.concatenate([av_sum[b][:, msl], ones_mq, xB[b][:, msl]], 0)
        in_maps2.append({
            "avs": np.concatenate([wblk, rhs], 1).astype(BF16),
        })
    res2 = run_bass_kernel_spmd(p2, in_maps2, list(range(NCORES)))

    out = np.zeros((B, C, N), np.float32)
    for core in range(NCORES):
        b, mq = divmod(core, 4)
        out[b][:, mq * MQ:(mq + 1) * MQ] = res2.results[core]["outc"]
    return out.reshape(B, C, H, W)
